# revision 1
# baseline (speedup 1.0000x reference)
"""Trainium2 kernel for: out = tanh(x @ scatter_nd(nonzero_ind, kernel_vector, (20000, 4096)) + bias).

Strategy (8 NeuronCores), W-resident / x-streaming:
  - Host builds the dense (20000, 4096) weight matrix from the COO triples.
  - Shard: units x8  ->  core c owns W[:, c*512:(c+1)*512] and computes
    out[:, c*512:(c+1)*512] = x @ W_c for the FULL batch.  No K-split, so the
    contraction is the minimal 157 k-tiles (20096 rows, 96 rows zero pad) and
    no cross-core partial summation is needed.
  - On device: the core's W slice (20096 x 512 fp16, 20.6 MB) is SBUF-resident
    as 157 [128 x 512] tiles, loaded once during the first batch chunk's
    k-loop, alternating between the sync and scalar hardware DGE queues
    (~38 GB/s each).  x^T streams on the gpsimd SWDGE queue as [128 x 1024]
    fp16 tiles (two 512-row batch blocks per DMA, 151 GB/s sustained) -- the
    software DGE queue sustains this without the completion-semaphore
    stalls the hardware queues exhibit.  Every x tile is consumed by 8
    matmuls (4 W subtiles x 2 batch blocks), with each W subtile stationary
    for 2 consecutive matmuls.  PSUM: 4 usub x 2 bblock = 8 banks, accumulated
    over all 157 k-tiles; batch is processed in 2 chunks of 1024 rows.
    Output (transposed [128u x 512b] bf16 blocks) leaves on the sync/scalar
    queues -- draining the SWDGE queue at kernel end costs ~8 us, so outs
    stay off gpsimd.
  - Host reassembles/transposes the output blocks, adds bias, applies tanh.

Per core: 2512 matmuls ([128x128] stationary x [128x512] moving, ~216.8 ns
steady) ~= 545 us PE time; measured ~569 us HW exec (vs 575.5 us baseline).
"""

import numpy as np

P = 128
B, K, U = 2048, 20000, 4096
USPLIT = 8
KT = 157                 # k-tiles (full contraction per core)
KPAD = KT * P            # 20096 rows (96 zero pad)
U_SH = U // USPLIT       # 512 unit cols per core
NUS = U_SH // P          # 4 W subtiles (stationary blocks) per k-tile
BBLK = 512               # moving free dim per matmul (batch block)
NCH = 2                  # batch chunks
BCH = B // NCH           # 1024 batch rows per chunk
NBB = BCH // BBLK        # 2 batch blocks per chunk -> 4*2 = 8 PSUM banks

TRACE = False            # set by test harness for profiled runs
LAST_RESULT = None       # BassKernelResults of the last run (for the harness)

_NC_CACHE = {}


def _build_nc():
    from concourse import bacc
    import concourse.mybir as mybir
    import concourse.tile as tile

    f32 = mybir.dt.float32
    f16 = mybir.dt.float16

    nc = bacc.Bacc("TRN2", target_bir_lowering=False, debug=False)
    # x^T tiles: [chunk, kt, p, s*512+b] = x[chunk*1024 + s*512 + b, kt*128+p]
    xt_d = nc.dram_tensor("xt_sh", [NCH, KT, P, 2 * BBLK], f16,
                          kind="ExternalInput").ap()
    w_d = nc.dram_tensor("w_sh", [KT, P, U_SH], f16, kind="ExternalInput").ap()
    # out blocks: [chunk, usub, s, p(u), b], bf16 (host upcasts; z values are
    # O(1-6) so bf16 rounding adds <~2e-3 abs err vs the 2e-2 budget)
    bf16 = mybir.dt.bfloat16
    o_d = nc.dram_tensor("out_p", [NCH, NUS, NBB, P, BBLK], bf16,
                         kind="ExternalOutput").ap()

    with tile.TileContext(nc) as tc:
        with (
            tc.tile_pool(name="resid", bufs=1) as respool,
            tc.tile_pool(name="xpool", bufs=8) as xpool,
            tc.tile_pool(name="stage", bufs=4) as spool,
            tc.tile_pool(name="mpsum", bufs=1, space="PSUM") as mpsum,
        ):
            # Resident W tiles; DMAs interleaved with chunk 0's k-loop.
            wres = [
                respool.tile([P, U_SH], f16, tag=f"w{kt}", name=f"w{kt}")
                for kt in range(KT)
            ]

            # out[u, b] += W[k, u].T @ xT[k, b], accumulated over all k-tiles
            # in PSUM bank (usub, bblock).  x tiles and resident-W tiles
            # alternate between the two hardware DGE queues (sync/scalar):
            # chunk 0 moves x (151 GB/s) + W (76 GB/s), which one queue alone
            # cannot sustain -- and any PE stall also costs ~3 us of mid
            # p-state afterwards.
            for ch in range(NCH):
                psums = [
                    [
                        mpsum.tile([P, BBLK], f32, tag=f"ps{us}_{s}",
                                   name=f"ps{us}_{s}")
                        for s in range(NBB)
                    ]
                    for us in range(NUS)
                ]
                for kt in range(KT):
                    if ch == 0:
                        # W-resident split across both HW DGE queues (38 GB/s
                        # each); the latency-critical x stream rides gpsimd's
                        # SWDGE queue, which showed none of the HWDGE
                        # completion-semaphore stalls.
                        weng = nc.sync if kt % 2 == 0 else nc.scalar
                        weng.dma_start(wres[kt][:], w_d[kt])
                    xs = xpool.tile([P, 2 * BBLK], f16, tag="xs", name="xs")
                    nc.gpsimd.dma_start(xs[:], xt_d[ch, kt])
                    for us in range(NUS):
                        for s in range(NBB):
                            nc.tensor.matmul(
                                psums[us][s][:],
                                wres[kt][:, us * P:(us + 1) * P],
                                xs[:, s * BBLK:(s + 1) * BBLK],
                                start=(kt == 0),
                                stop=(kt == KT - 1),
                            )
                for us in range(NUS):
                    for s in range(NBB):
                        st = spool.tile([P, BBLK], bf16, tag="st", name="st")
                        nc.vector.tensor_copy(st[:], psums[us][s][:])
                        # Keep outs off gpsimd: draining the SWDGE queue at
                        # kernel end costs ~8us.
                        oeng = nc.sync if (us * NBB + s) % 2 == 0 else nc.scalar
                        oeng.dma_start(o_d[ch, us, s], st[:])

    nc.compile()
    return nc


def _get_nc():
    if "nc" not in _NC_CACHE:
        _NC_CACHE["nc"] = _build_nc()
    return _NC_CACHE["nc"]


def kernel(x, kernel_vector, bias, nonzero_ind):
    global LAST_RESULT
    from concourse.bass_utils import run_bass_kernel_spmd

    x = np.asarray(x, dtype=np.float32)
    kernel_vector = np.asarray(kernel_vector, dtype=np.float32)
    bias = np.asarray(bias, dtype=np.float32)
    nonzero_ind = np.asarray(nonzero_ind)

    nc = _get_nc()

    # Host scatter: dense weights [KPAD, U] fp16 (rows >= 20000 stay zero).
    rows = nonzero_ind[:, 0].astype(np.int64)
    cols = nonzero_ind[:, 1].astype(np.int64)
    w_full = np.zeros(KPAD * U, np.float32)
    np.add.at(w_full, rows * U + cols, kernel_vector)
    w_full = w_full.reshape(KPAD, U).astype(np.float16)

    # x^T arranged [NCH, KT, 128, 2*BBLK]; shared by all cores.
    x16 = x.astype(np.float16)
    xt = np.zeros((KPAD, B), np.float16)
    xt[:K] = x16.T
    # [KT, 128, NCH, NBB, BBLK] -> [NCH, KT, 128, NBB*BBLK]
    xt = np.ascontiguousarray(
        xt.reshape(KT, P, NCH, NBB * BBLK).transpose(2, 0, 1, 3)
    )

    in_maps = []
    for c in range(USPLIT):
        w_sh = np.ascontiguousarray(
            w_full[:, c * U_SH:(c + 1) * U_SH]
        ).reshape(KT, P, U_SH)
        in_maps.append({"xt_sh": xt, "w_sh": w_sh})

    kwargs = {}
    if TRACE:
        kwargs = dict(trace=True, trace_cores=list(range(8)))
    res = run_bass_kernel_spmd(nc, in_maps, core_ids=list(range(8)), **kwargs)
    LAST_RESULT = res

    out = np.empty((B, U), np.float32)
    for c in range(USPLIT):
        # [NCH, NUS, NBB, P, BBLK] -> [NCH, NBB, BBLK, NUS, P] -> [B, U_SH]
        blk = (
            res.results[c]["out_p"]
            .astype(np.float32)
            .transpose(0, 2, 4, 1, 3)
            .reshape(B, U_SH)
        )
        out[:, c * U_SH:(c + 1) * U_SH] = blk
    out += bias[None, :]
    np.tanh(out, out=out)
    return out

